# revision 16
# baseline (speedup 1.0000x reference)
"""Bass/Trainium2 kernel for bidirectional cross-attention.

Computes, per batch b:
    S    = image @ text^T * D**-0.5          [Ni, Nt]
    P    = softmax(S, axis=-1)
    image_out = P @ text                     [Ni, D]
    text_out  = P^T @ image                  [Nt, D]

Sharding: batch (4) x image-row-half (2) -> 8 cores. text replicated per
batch pair; text_out partials summed on host (one partial per core).

Per-core algorithm (R=2048 image rows, Nt=4096, D=256), all PE matmuls
in bf16 (full-rate PE, inputs ~N(0,1) so bf16 rounding stays ~1e-3):
  - Load I [R,D], T [Nt,D] fp32; PE-transpose into bf16 Id [D,R],
    Td [D,Nt] (contraction over D needs D on partitions). T_mm2 = bf16
    cast of T (img-matmul rhs).
  - For each superblock of 512 image rows:
      Phase A: S rows via matmul; exp(S*scale) -> expS bf16 [i, t],
               fused accum_out giving softmax denominators s.
               I' = I * (1/s)  (defers normalization of text_out).
      Phase B: per pair of text tiles, PE-transpose expS -> expST bf16
               (PSUM, one bank), stage to SBUF via Vector; accumulate
               text_out over i in PSUM using expS as lhsT and I' as rhs,
               drain-add into SBUF accumulator txt_acc; accumulate
               image_out over t in PSUM using expST as lhsT.
      image_out normalized by 1/s during PSUM drain (ACT scaled copy).
      txt_acc streamed to DRAM during the last superblock.
Softmax max-subtraction is skipped: scores ~ N(0,1), exp range is safe
in fp32 and matches jax softmax to ~1e-7.
"""

import numpy as np
from contextlib import ExitStack

import concourse.bass as bass
import concourse.tile as tile
from concourse import bacc, mybir
from concourse.bass_utils import run_bass_kernel_spmd
from concourse.masks import make_identity

P = 128
D = 256
B = 4
N_FULL = 4096  # image/text tokens per batch
N_CORES = 8
R = 2048  # image rows per core (N_FULL / 2)
SCALE = float(D) ** -0.5

F32 = mybir.dt.float32
F32R = mybir.dt.float32r
BF16 = mybir.dt.bfloat16
EXP = mybir.ActivationFunctionType.Exp
COPY = mybir.ActivationFunctionType.Copy


def build_nc():
    rows, ntext = R, N_FULL
    i_tiles = rows // P    # 16
    t_tiles = ntext // P   # 32
    sb_i = 2               # i-tiles per superblock
    n_sb = i_tiles // sb_i  # 8
    sb_rows = sb_i * P     # 512
    n_c2 = ntext // 1024   # 4 (1024-wide column blocks in phase A)

    nc = bacc.Bacc("TRN2", target_bir_lowering=False, debug=False,
                   num_devices=N_CORES)
    img = nc.dram_tensor("img", [rows, D], F32R, kind="ExternalInput").ap()
    txt = nc.dram_tensor("txt", [ntext, D], F32R, kind="ExternalInput").ap()
    img_out = nc.dram_tensor("img_out", [rows, D], F32,
                             kind="ExternalOutput").ap()
    txt_part = nc.dram_tensor("txt_part", [ntext, D], F32,
                              kind="ExternalOutput").ap()

    with tile.TileContext(nc) as tc:
        with ExitStack() as ctx:
            const = ctx.enter_context(tc.tile_pool(name="const", bufs=1))
            T_nat = const.tile([P, t_tiles, D], F32R)
            I_nat = const.tile([P, i_tiles, D], F32R)
            T_mm2 = const.tile([P, t_tiles, D], BF16)
            Td = const.tile([P, 2, ntext], BF16)
            Id = const.tile([P, 2, rows], BF16)
            I_mm2 = const.tile([P, i_tiles, D], BF16)
            ident = const.tile([P, P], F32)
            ident_r = const.tile([P, P], F32R)
            ident_b = const.tile([P, P], BF16)
            rs = const.tile([P, i_tiles], F32)
            ssum = const.tile([P, sb_i * n_c2], F32)
            txt_acc = const.tile([P, t_tiles, D], F32)
            img_sb = const.tile([P, sb_i, D], F32)

            # Image DMA first (phase A needs Id before anything else), then
            # text in 8-tile chunks so Td transposes can start before the
            # whole text tensor lands.
            img_r = img.rearrange("(i p) d -> p i d", p=P)
            nc.sync.dma_start(I_nat[:, :4, :], img_r[:, :4, :])
            nc.sync.dma_start(I_nat[:, 4:, :], img_r[:, 4:, :])
            txt_r = txt.rearrange("(t p) d -> p t d", p=P)
            for g in range(t_tiles // 8):
                nc.sync.dma_start(T_nat[:, g * 8:(g + 1) * 8, :],
                                  txt_r[:, g * 8:(g + 1) * 8, :])
            make_identity(nc, ident[:])
            nc.vector.tensor_copy(ident_r[:], ident[:])
            nc.vector.tensor_copy(ident_b[:], ident[:])

            ps_work = ctx.enter_context(
                tc.tile_pool(name="ps_work", bufs=2, space="PSUM"))
            ps_tp = ctx.enter_context(
                tc.tile_pool(name="ps_tp", bufs=2, space="PSUM"))
            ps_img = ctx.enter_context(
                tc.tile_pool(name="ps_img", bufs=sb_i // 2, space="PSUM"))
            ps_txt = ctx.enter_context(
                tc.tile_pool(name="ps_txt", bufs=1, space="PSUM"))
            expS_pool = ctx.enter_context(tc.tile_pool(name="expS", bufs=2))
            est_pool = ctx.enter_context(tc.tile_pool(name="est", bufs=2))
            small = ctx.enter_context(tc.tile_pool(name="small", bufs=4))

            def emit_a_chunk(sb, iil, c2, expS):
                """One 1024-wide column chunk of S for (superblock sb,
                i-tile iil) -> exp into expS, accumulating partial row sums."""
                ii = sb * sb_i + iil
                ps = ps_work.tile([P, 1024], F32, name="ps", tag="psw")
                for half in range(2):
                    c0 = c2 * 1024 + half * 512
                    for k in range(2):
                        nc.tensor.matmul(
                            ps[:, half * 512:(half + 1) * 512],
                            Id[:, k, ii * P:(ii + 1) * P],
                            Td[:, k, c0:c0 + 512],
                            start=(k == 0), stop=(k == 1))
                nc.scalar.activation(
                    expS[:, iil, c2 * 1024:(c2 + 1) * 1024],
                    ps[:], EXP, scale=SCALE,
                    accum_out=ssum[:, iil * n_c2 + c2:iil * n_c2 + c2 + 1])

            def emit_a_fin(sb, iil):
                """Finish i-tile iil of superblock sb: softmax denominator
                and the 1/s-scaled bf16 image rows for the text matmuls."""
                ii = sb * sb_i + iil
                srow = small.tile([P, 1], F32)
                nc.vector.reduce_sum(
                    srow[:], ssum[:, iil * n_c2:(iil + 1) * n_c2],
                    axis=mybir.AxisListType.X)
                nc.vector.reciprocal(rs[:, ii:ii + 1], srow[:])
                nc.vector.tensor_scalar_mul(
                    I_mm2[:, ii, :], I_nat[:, ii, :], rs[:, ii:ii + 1])

            # ---- init + prologue, interleaved to keep PE continuously fed:
            # Id transpose groups first (gated only by the image DMA), then
            # per 8-text-tile group: bf16 cast (Vector), Td transposes (PE,
            # bf16), and phase A chunks of superblock 0 at that c2.
            expS_cur = expS_pool.tile([P, sb_i, ntext], BF16, name="expS",
                                      tag="expS")
            for g in range(i_tiles // 4):
                pt = ps_work.tile([P, 1024], F32R, name="pid", tag="psw")
                for k in range(2):
                    for j in range(4):
                        nc.tensor.transpose(
                            pt[:, k * 512 + j * P:k * 512 + (j + 1) * P],
                            I_nat[:, g * 4 + j, k * P:(k + 1) * P],
                            ident_r[:])
                for k in range(2):
                    nc.vector.tensor_copy(Id[:, k, g * 512:(g + 1) * 512],
                                          pt[:, k * 512:(k + 1) * 512])
            for g in range(t_tiles // 8):
                nc.vector.tensor_copy(T_mm2[:, g * 8:(g + 1) * 8, :],
                                      T_nat[:, g * 8:(g + 1) * 8, :])
                ptd = ps_work.tile([P, 2048], BF16, name="ptd", tag="psw")
                for k in range(2):
                    for j in range(8):
                        nc.tensor.transpose(
                            ptd[:, k * 1024 + j * P:k * 1024 + (j + 1) * P],
                            T_mm2[:, g * 8 + j, k * P:(k + 1) * P],
                            ident_b[:])
                for k in range(2):
                    nc.vector.tensor_copy(Td[:, k, g * 1024:(g + 1) * 1024],
                                          ptd[:, k * 1024:(k + 1) * 1024])
                for iil in range(sb_i):
                    emit_a_chunk(0, iil, g, expS_cur)
            for iil in range(sb_i):
                emit_a_fin(0, iil)

            for sb in range(n_sb):
                # Phase A of sb+1 is interleaved into phase B of sb (one
                # chunk per t2 iteration) so the exps hide under B's PE work.
                if sb < n_sb - 1:
                    expS_next = expS_pool.tile([P, sb_i, ntext], BF16, name="expS", tag="expS")
                expS = expS_cur

                # Image accumulators packed two per psum bank (one live
                # accumulation group per bank: only the first matmul of the
                # bank carries start=True).
                pimg = [ps_img.tile([P, 2 * D], F32, name=f"pimg{x}",
                                    tag="pimg") for x in range(sb_i // 2)]
                for t2 in range(t_tiles // 2):
                    # expST for a pair of text tiles: 8 PE transposes into
                    # one bf16 PSUM bank, staged to SBUF by Vector.
                    tp = ps_tp.tile([P, 2 * sb_i * P], BF16)
                    for half in range(2):
                        t = 2 * t2 + half
                        for iil in range(sb_i):
                            nc.tensor.transpose(
                                tp[:, (half * sb_i + iil) * P:
                                   (half * sb_i + iil + 1) * P],
                                expS[:, iil, t * P:(t + 1) * P],
                                ident_b[:])
                    est = est_pool.tile([P, 2 * sb_i * P], BF16)
                    nc.vector.tensor_copy(est[:], tp[:])

                    # text_out partials first (independent of est copy, so
                    # PE never waits on the Vector stage).
                    ptxt = ps_txt.tile([P, 2 * D], F32)
                    for half in range(2):
                        t = 2 * t2 + half
                        for iil in range(sb_i):
                            nc.tensor.matmul(
                                ptxt[:, half * D:(half + 1) * D],
                                expS[:, iil, t * P:(t + 1) * P],
                                I_mm2[:, sb * sb_i + iil, :],
                                start=(half == 0 and iil == 0),
                                stop=(half == 1 and iil == sb_i - 1),
                                skip_group_check=True)
                    if sb == 0:
                        nc.vector.tensor_copy(
                            txt_acc[:, 2 * t2:2 * t2 + 2, :], ptxt[:])
                    else:
                        nc.vector.tensor_add(
                            txt_acc[:, 2 * t2:2 * t2 + 2, :],
                            txt_acc[:, 2 * t2:2 * t2 + 2, :], ptxt[:])

                    for half in range(2):
                        t = 2 * t2 + half
                        for iil in range(sb_i):
                            nc.tensor.matmul(
                                pimg[iil // 2][:, (iil % 2) * D:(iil % 2 + 1) * D],
                                est[:, (half * sb_i + iil) * P:
                                    (half * sb_i + iil + 1) * P],
                                T_mm2[:, t, :],
                                start=(t2 == 0 and half == 0 and iil % 2 == 0),
                                stop=(t2 == t_tiles // 2 - 1 and half == 1
                                      and iil % 2 == 1),
                                skip_group_check=True)

                    if sb < n_sb - 1 and t2 % 2 == 0:
                        iil_a, c2_a = divmod(t2 // 2, n_c2)
                        emit_a_chunk(sb + 1, iil_a, c2_a, expS_next)
                        if c2_a == n_c2 - 1:
                            emit_a_fin(sb + 1, iil_a)

                    # stream text_out during the last superblock
                    if sb == n_sb - 1 and t2 % 2 == 1:
                        tt0 = 2 * (t2 - 1)
                        nc.sync.dma_start(
                            txt_part[tt0 * P:(tt0 + 4) * P, :].rearrange(
                                "(t p) d -> p t d", p=P),
                            txt_acc[:, tt0:tt0 + 4, :])

                # ---- drain image_out, normalized by 1/s ----
                for iil in range(sb_i):
                    nc.scalar.activation(
                        img_sb[:, iil, :],
                        pimg[iil // 2][:, (iil % 2) * D:(iil % 2 + 1) * D],
                        COPY,
                        scale=rs[:, sb * sb_i + iil:sb * sb_i + iil + 1])
                nc.sync.dma_start(
                    img_out[sb * sb_rows:(sb + 1) * sb_rows, :].rearrange(
                        "(ii p) d -> p ii d", p=P),
                    img_sb[:])
                if sb < n_sb - 1:
                    expS_cur = expS_next

    nc.compile()
    return nc


_CACHE = {}


def _get_nc():
    if "nc" not in _CACHE:
        _CACHE["nc"] = build_nc()
    return _CACHE["nc"]


def kernel(image_features, text_features):
    image_features = np.asarray(image_features, dtype=np.float32)
    text_features = np.asarray(text_features, dtype=np.float32)
    nc = _get_nc()

    in_maps = []
    for c in range(N_CORES):
        b, h = divmod(c, 2)
        in_maps.append({
            "img": np.ascontiguousarray(
                image_features[b, h * R:(h + 1) * R, :]),
            "txt": np.ascontiguousarray(text_features[b]),
        })
    res = run_bass_kernel_spmd(nc, in_maps, core_ids=list(range(N_CORES))).results

    image_out = np.empty((B, N_FULL, D), np.float32)
    text_out = np.empty((B, N_FULL, D), np.float32)
    for c in range(N_CORES):
        b, h = divmod(c, 2)
        image_out[b, h * R:(h + 1) * R, :] = res[c]["img_out"]
    for b in range(B):
        text_out[b] = res[2 * b]["txt_part"] + res[2 * b + 1]["txt_part"]
    return image_out, text_out
